# revision 1
# baseline (speedup 1.0000x reference)
# Trainium2 Bass kernel for nn_CapsuleLayer (dynamic-routing capsule layer).
#
# Math reformulation (exact, not approximate):
#   einsum('bni,njkl->bnjl', x, W) contracts i and k independently, so
#     predictions[b,n,j,l] = xs[b,n] * Ws[n,j,l]
#   with xs = x.sum(-1) [B,N], Ws = W.sum(2) [N,J,L].
#   Routing logits b start at 0 and update b += xs * (Ws . v), which is linear
#   in v, so the [B,N,J] logit state never needs to be materialized:
#     c_i = softmax_j( xs[b,n] * (Ws[n,j,:] . Vacc[b,j,:]) ),  Vacc = sum of past v's
#   Each iteration:
#     u[b,n,j] = xs[b,n] * c[b,n,j]
#     s[b,j,l] = sum_n u[b,n,j] * Ws[n,j,l]      (PE cross-product + diag extract)
#     v = squash(s)
#
# Sharding: batch B=64 split over 8 cores (8 samples each), W replicated.
# Per-core HBM traffic is dominated by the 37.7MB W read (memory-bound).

import numpy as np

import concourse.bass as bass
import concourse.mybir as mybir
import concourse.tile as tile
from concourse import bacc
from concourse import bass_utils
from concourse.bass import ts
from concourse.masks import make_identity

B, N, DI, J, L = 64, 2304, 8, 32, 16
NCORES = 8
BC = B // NCORES          # 8 samples per core
P = 128
NT = N // P               # 18 n-tiles
JL = J * L                # 512
BJ = BC * J               # 256
NG = JL // P              # 4 partition groups of the (j,l) axis
EPS = 1e-7

F32 = mybir.dt.float32
F32R = mybir.dt.float32r
AX = mybir.AxisListType
AF = mybir.ActivationFunctionType
ALU = mybir.AluOpType


def r(ap):
    """bitcast to float32r for full-rate PE matmuls (moving dim >= 256)."""
    return ap.bitcast(F32R)


def _emit(ctx, tc, x_ap, w_ap, v_ap, stage=5):
    nc = tc.nc

    # ---------------- pools ----------------
    const = ctx.enter_context(tc.tile_pool(name="const", bufs=1))
    wstream = ctx.enter_context(tc.tile_pool(name="wstream", bufs=4))
    xio = ctx.enter_context(tc.tile_pool(name="xio", bufs=2))
    work = ctx.enter_context(tc.tile_pool(name="work", bufs=4))
    small = ctx.enter_context(tc.tile_pool(name="small", bufs=4))
    tailp = ctx.enter_context(tc.tile_pool(name="tailp", bufs=2))
    ps_xs = ctx.enter_context(tc.tile_pool(name="ps_xs", bufs=1, space="PSUM"))
    ps_wT = ctx.enter_context(tc.tile_pool(name="ps_wT", bufs=1, space="PSUM"))
    ps_A = ctx.enter_context(tc.tile_pool(name="ps_A", bufs=3, space="PSUM"))
    ps_S = ctx.enter_context(tc.tile_pool(name="ps_S", bufs=1, space="PSUM"))
    ps_vT = ctx.enter_context(tc.tile_pool(name="ps_vT", bufs=1, space="PSUM"))

    # ---------------- constants ----------------
    id128 = const.tile([P, P], F32, name="id128")
    make_identity(nc, id128)
    id128r = const.tile([P, P], F32R, name="id128r")
    nc.vector.tensor_copy(out=id128r, in_=id128)
    id8 = const.tile([BC, BC], F32, name="id8")
    make_identity(nc, id8)

    # Mdiag[p, 0, k] = 1.0 iff k == p % 32   (diagonal-block extraction mask)
    Mdiag = const.tile([P, 1, 1, J], F32, name="Mdiag")
    nc.gpsimd.memset(Mdiag, 0.0)
    for q in range(P // J):
        nc.gpsimd.affine_select(
            out=Mdiag[ts(q, J), 0, 0, :],
            in_=Mdiag[ts(q, J), 0, 0, :],
            compare_op=ALU.not_equal,
            fill=1.0,
            base=0,
            pattern=[[-1, J]],
            channel_multiplier=1,
        )

    # maskbd[p, g, j] = 1.0 iff j == g*8 + p//16  (block-diag v builder),
    # i.e. iff (p + 128g - 16j) in [0, 15]. Built arithmetically because
    # engine ops require 32-aligned partition starts.
    jiota = const.tile([P, 1, J], F32, name="jiota")
    nc.gpsimd.iota(
        out=jiota,
        pattern=[[0, 1], [1, J]],
        base=0,
        channel_multiplier=0,
        allow_small_or_imprecise_dtypes=True,
    )
    piota = const.tile([P, 1, 1], F32, name="piota")
    nc.gpsimd.iota(
        out=piota,
        pattern=[[0, 1]],
        base=0,
        channel_multiplier=1,
        allow_small_or_imprecise_dtypes=True,
    )
    maskbd = const.tile([P, NG, 1, J], F32, name="maskbd")
    pg = const.tile([P, 1, 1], F32, name="pg")
    q_t = const.tile([P, 1, J], F32, name="q_t")
    qa = const.tile([P, 1, J], F32, name="qa")
    qb = const.tile([P, 1, J], F32, name="qb")
    for g in range(NG):
        nc.vector.tensor_scalar_add(out=pg, in0=piota, scalar1=float(P * g))
        nc.vector.tensor_scalar(
            out=q_t, in0=jiota, scalar1=-float(L), scalar2=pg,
            op0=ALU.mult, op1=ALU.add,
        )
        nc.vector.tensor_scalar(
            out=qa, in0=q_t, scalar1=0.0, scalar2=None, op0=ALU.is_ge
        )
        nc.vector.tensor_scalar(
            out=qb, in0=q_t, scalar1=float(L - 1), scalar2=None, op0=ALU.is_le
        )
        nc.vector.tensor_mul(out=maskbd[:, g], in0=qa, in1=qb)

    eps_ap = const.tile([P, 1], F32, name="eps_ap")
    nc.gpsimd.memset(eps_ap, EPS)

    # REP[p, (jm l)] = 1.0 iff l == p  (partition-replication stationary matrix:
    # out = REP.T @ rhs copies rhs's 16 partitions to all 8 16-partition groups)
    REP = const.tile([L, P], F32, name="REP")
    nc.gpsimd.memset(REP, 0.0)
    nc.gpsimd.affine_select(
        out=REP.rearrange("p (jm l) -> p jm l", l=L),
        in_=REP.rearrange("p (jm l) -> p jm l", l=L),
        compare_op=ALU.not_equal,
        fill=1.0,
        base=0,
        pattern=[[0, P // L], [1, L]],
        channel_multiplier=-1,
    )
    REPr = const.tile([L, P], F32R, name="REPr")
    nc.vector.tensor_copy(out=REPr, in_=REP)

    # ---------------- persistent tensors ----------------
    xs = const.tile([P, NT, BC, 1], F32, name="xs")        # xs[n%128, n//128, b]
    Ws = const.tile([P, NT, JL], F32R, name="Ws")           # Ws[n%128, n//128, (j l)]
    WsT = const.tile([P, NG, N], F32R, name="WsT")          # WsT[(j l)%128, (j l)//128, n]
    Vrep = const.tile([P, BJ], F32, name="Vrep")           # Vacc[l, (b j)] replicated x8 over partitions

    # ---------------- x prep: xs[n, b] = sum_i x[b, n, i], transposed ----------------
    def x_prep():
        xs_ps = ps_xs.tile([P, NT, BC, 1], F32, name="xs_ps")
        for t in range(NT):
            x_t = xio.tile([BC, P, DI], F32, name="x_t", tag="x_t")
            nc.sync.dma_start(out=x_t, in_=x_ap[:, ts(t, P), :])
            xsb_t = xio.tile([BC, P], F32, name="xsb_t", tag="xsb_t")
            nc.vector.reduce_sum(out=xsb_t, in_=x_t, axis=AX.X)
            nc.tensor.transpose(out=xs_ps[:, t, :, 0], in_=xsb_t, identity=id8)
        nc.vector.tensor_copy(out=xs, in_=xs_ps)
        return xs_ps

    # ---------------- W stream: k-reduce -> Ws, transpose -> WsT ----------------
    def w_phase_tile(t):
        w_t = wstream.tile([P, J, DI, L], F32, name="w_t", tag="w_t")
        JH = J // 2
        for half in range(2):
            wh = w_t[:, half * JH : (half + 1) * JH, :, :]
            nc.sync.dma_start(
                out=wh, in_=w_ap[ts(t, P), half * JH : (half + 1) * JH]
            )
            # k-reduction: DVE does a strided-view reduce; POOL (which cannot
            # do free-axis reduces) gets an in-place add-tree on its share.
            if (t % 3 == 2 and t != 17) or t in (13, 16):
                nc.gpsimd.tensor_add(
                    out=wh[:, :, 0 : DI // 2, :],
                    in0=wh[:, :, 0 : DI // 2, :],
                    in1=wh[:, :, DI // 2 : DI, :],
                )
                nc.gpsimd.tensor_add(
                    out=wh[:, :, 0 : DI // 4, :],
                    in0=wh[:, :, 0 : DI // 4, :],
                    in1=wh[:, :, DI // 4 : DI // 2, :],
                )
                nc.gpsimd.tensor_add(
                    out=Ws[:, t, half * JH * L : (half + 1) * JH * L].rearrange(
                        "p (j l) -> p j l", j=JH
                    ),
                    in0=wh[:, :, 0, :],
                    in1=wh[:, :, 1, :],
                )
            else:
                nc.vector.reduce_sum(
                    out=Ws[:, t, half * JH * L : (half + 1) * JH * L].rearrange(
                        "p (j l) -> p j l", j=JH
                    ),
                    in_=wh.rearrange("p j k l -> p j l k"),
                    axis=AX.X,
                )
        wT_ps = ps_wT.tile([P, JL], F32R, name="wT_ps", tag="ps_share")
        for g in range(NG):
            nc.tensor.transpose(
                out=wT_ps[:, ts(g, P)], in_=Ws[:, t, ts(g, P)], identity=id128r
            )
        nc.scalar.copy(
            out=WsT[:, :, ts(t, P)],
            in_=wT_ps.rearrange("p (g n) -> p g n", g=NG),
        )

    # ---------------- one routing iteration ----------------
    def routing_iter(it, bd, xs_ps, sub=9):
        """it in {1,2,3}; bd is the block-diag Vacc tensor (None for it==1).
        Returns v_a [P, 2, L] where row p of half h holds v[b, j, :] with
        b = 4*h + p//32, j = p % 32."""
        psS = [
            ps_S.tile([P, JL], F32, name=f"psS{h}_{it}", tag=f"psS{h}")
            for h in range(2)
        ]
        for t in range(NT):
            if it == 1:
                # c is uniform 1/J: u = xs / J, broadcast over j.
                u = work.tile([P, BJ], F32R, name="u", tag="u")
                nc.vector.tensor_scalar(
                    out=u.rearrange("p (b j) -> p b j", b=BC),
                    in0=xs_ps[:, t].to_broadcast([P, BC, J]),
                    scalar1=1.0 / J,
                    scalar2=None,
                    op0=ALU.mult,
                )
            else:
                psA = ps_A.tile([P, BC, J], F32, name="psA", tag="psA")
                for g in range(NG):
                    nc.tensor.matmul(
                        psA,
                        lhsT=WsT[:, g, ts(t, P)],
                        rhs=bd[:, g, :, :],
                        start=(g == 0),
                        stop=(g == NG - 1),
                    )
                # logits = xs * A ; c = softmax_j ; u = xs * c
                Lt = work.tile([P, BJ], F32, name="Lt", tag="Lt")
                nc.vector.tensor_mul(
                    out=Lt.rearrange("p (b j) -> p b j", b=BC),
                    in0=psA,
                    in1=xs[:, t].to_broadcast([P, BC, J]),
                )
                Et = work.tile([P, BJ], F32, name="Et", tag="Et")
                nc.scalar.activation(out=Et, in_=Lt, func=AF.Exp)
                St = small.tile([P, BC, 1], F32, name="St", tag="St")
                nc.vector.reduce_sum(
                    out=St, in_=Et.rearrange("p (b j) -> p b j", b=BC), axis=AX.X
                )
                Rt = small.tile([P, BC, 1], F32, name="Rt", tag="Rt")
                nc.vector.reciprocal(out=Rt, in_=St)
                xsR = small.tile([P, BC, 1], F32, name="xsR", tag="xsR")
                nc.vector.tensor_mul(out=xsR, in0=Rt, in1=xs[:, t])
                u = work.tile([P, BJ], F32R, name="u", tag="u")
                nc.gpsimd.tensor_mul(
                    out=u.rearrange("p (b j) -> p b j", b=BC),
                    in0=Et.rearrange("p (b j) -> p b j", b=BC),
                    in1=xsR.to_broadcast([P, BC, J]),
                )
            for h in range(2):
                nc.tensor.matmul(
                    psS[h],
                    lhsT=u[:, ts(h, P)],
                    rhs=Ws[:, t, :],
                    start=(t == 0),
                    stop=(t == NT - 1),
                )

        # ---- diagonal extraction: s[p, h, l] from psS[h][(b j), (j' l)] ----
        s_a = tailp.tile([P, 2, L], F32, name="s_a", tag="s_a")
        if sub == 1:
            for h in range(2):
                nc.vector.tensor_copy(out=s_a[:, h, :], in_=psS[h][:, 0:L])
            return s_a.bitcast(F32R)
        for h in range(2):
            dtmp = tailp.tile([P, L, J], F32, name="dtmp", tag="dtmp")
            nc.vector.tensor_mul(
                out=dtmp,
                in0=psS[h].rearrange("p (k l) -> p l k", k=J),
                in1=Mdiag[:, 0].to_broadcast([P, L, J]),
            )
            nc.vector.reduce_sum(out=s_a[:, h, :], in_=dtmp, axis=AX.X)

        if sub == 2:
            return s_a.bitcast(F32R)

        # ---- squash: v = s * n/(1+n)/sqrt(n+eps) ----
        nrm = tailp.tile([P, 2, 1], F32, name="nrm", tag="nrm")
        sq = tailp.tile([P, 2, L], F32, name="sq", tag="sq")
        nc.vector.tensor_mul(out=sq, in0=s_a, in1=s_a)
        if sub == 6:
            return sq.bitcast(F32R)
        nc.vector.reduce_sum(out=nrm, in_=sq, axis=AX.X)
        if sub == 7:
            nc.vector.tensor_scalar_add(out=sq[:, :, 0:1], in0=nrm, scalar1=0.0)
            return sq.bitcast(F32R)
        if sub == 3:
            nc.vector.tensor_copy(out=s_a[:, :, 0:1], in_=nrm)
            return s_a.bitcast(F32R)
        d1 = tailp.tile([P, 2, 1], F32, name="d1", tag="d1")
        nc.vector.tensor_scalar_add(out=d1, in0=nrm, scalar1=1.0)
        sqt = tailp.tile([P, 2, 1], F32, name="sqt", tag="sqt")
        nc.scalar.activation(out=sqt, in_=nrm, func=AF.Sqrt, bias=eps_ap)
        if sub == 4:
            nc.vector.tensor_copy(out=s_a[:, :, 0:1], in_=sqt)
            return s_a.bitcast(F32R)
        den = tailp.tile([P, 2, 1], F32, name="den", tag="den")
        nc.vector.tensor_mul(out=den, in0=d1, in1=sqt)
        rec = tailp.tile([P, 2, 1], F32, name="rec", tag="rec")
        nc.vector.reciprocal(out=rec, in_=den)
        # v = (s * rec) * nrm, fused per half via dual-scalar tensor_scalar
        v_a = tailp.tile([P, 2, L], F32R, name="v_a", tag="v_a")
        for h in range(2):
            nc.vector.tensor_scalar(
                out=v_a[:, h, :],
                in0=s_a[:, h, :],
                scalar1=rec[:, h, :],
                scalar2=nrm[:, h, :],
                op0=ALU.mult,
                op1=ALU.mult,
            )
        return v_a

    def accumulate_v(v_a, first):
        """Transpose v_a into vT[l, (b j)] (replicated over 8 partition groups)
        and accumulate into Vrep; build block-diag bd for the next iteration."""
        vT = ps_vT.tile([L, BJ], F32R, name="vT", tag="vT")
        for h in range(2):
            nc.tensor.transpose(
                out=vT[:, ts(h, P)], in_=v_a[:, h, :], identity=id128r
            )
        vT_sb = tailp.tile([L, BJ], F32R, name="vT_sb", tag="vT_sb")
        nc.vector.tensor_copy(out=vT_sb, in_=vT)
        vrep_ps = ps_wT.tile([P, BJ], F32, name="vrep_ps", tag="ps_share")
        nc.tensor.matmul(vrep_ps, lhsT=REPr, rhs=vT_sb, start=True, stop=True)
        if first:
            nc.vector.tensor_copy(out=Vrep, in_=vrep_ps)
        else:
            nc.vector.tensor_add(out=Vrep, in0=Vrep, in1=vrep_ps)
        bd = tailp.tile([P, NG, BC, J], F32R, name="bd", tag="bd")
        for g in range(NG):
            nc.gpsimd.tensor_mul(
                out=bd[:, g, :, :],
                in0=Vrep.rearrange("p (b j) -> p b j", b=BC),
                in1=maskbd[:, g].to_broadcast([P, BC, J]),
            )
        return bd

    # ---------------- main schedule ----------------
    v_flat = v_ap.rearrange("b j l -> (b j) l")

    def emit_out(v_x):
        for h in range(2):
            nc.sync.dma_start(out=v_flat[ts(h, P)], in_=v_x[:, h, :].bitcast(F32))

    def one_pass(sub=9):
        xs_ps = x_prep()
        if stage >= 2 or stage >= 100:
            for t in range(NT):
                w_phase_tile(t)
        if stage < 100:
            if stage < 3:
                stub = const.tile([P, 2, L], F32R, name="stub")
                nc.vector.tensor_scalar(
                    out=stub,
                    in0=xs_ps[:, 0, 0:1, :].to_broadcast([P, 2, L]),
                    scalar1=1.0,
                    scalar2=None,
                    op0=ALU.mult,
                )
                emit_out(stub)
                return
            v1 = routing_iter(1, None, xs_ps, sub=sub)
            if stage == 3 or 30 <= stage < 40:
                emit_out(v1)
                return
            bd1 = accumulate_v(v1, first=True)
            v2 = routing_iter(2, bd1, xs_ps)
            if stage == 4:
                emit_out(v2)
                return
            bd2 = accumulate_v(v2, first=False)
            v3 = routing_iter(3, bd2, xs_ps)
            emit_out(v3)
        else:
            v1 = routing_iter(1, None, xs_ps)
            bd1 = accumulate_v(v1, first=True)
            v2 = routing_iter(2, bd1, xs_ps)
            bd2 = accumulate_v(v2, first=False)
            v3 = routing_iter(3, bd2, xs_ps)
            emit_out(v3)

    if stage >= 100:
        for i in range(stage - 100):
            if i:
                tc.strict_bb_all_engine_barrier()
            one_pass()
    else:
        one_pass(sub=(stage - 30 if 30 <= stage < 40 else 9))


_nc_cache = {}


def build(stage=5):
    if stage not in _nc_cache:
        from contextlib import ExitStack

        nc = bacc.Bacc("TRN2", target_bir_lowering=False, debug=False)
        x_ap = nc.dram_tensor("x", [BC, N, DI], F32, kind="ExternalInput").ap()
        w_ap = nc.dram_tensor("w", [N, J, DI, L], F32, kind="ExternalInput").ap()
        v_ap = nc.dram_tensor("v", [BC, J, L], F32, kind="ExternalOutput").ap()
        with (
            tile.TileContext(nc) as tc,
            ExitStack() as ctx,
            nc.allow_low_precision(
                reason="f32r is a rounded fp32 view required for full-rate PE "
                "matmuls; accumulation still happens in fp32 PSUM"
            ),
        ):
            _emit(ctx, tc, x_ap, w_ap, v_ap, stage=stage)
        nc.compile()
        _nc_cache[stage] = nc
    return _nc_cache[stage]


def run(x, W, trace=False, trace_kwargs=None):
    x = np.ascontiguousarray(np.asarray(x, dtype=np.float32))
    W = np.ascontiguousarray(np.asarray(W, dtype=np.float32))
    assert x.shape == (B, N, DI) and W.shape == (N, J, DI, L)
    nc = build()
    in_maps = [
        {"x": x[i * BC : (i + 1) * BC], "w": W} for i in range(NCORES)
    ]
    res = bass_utils.run_bass_kernel_spmd(
        nc,
        in_maps,
        core_ids=list(range(NCORES)),
        trace=trace,
        **(trace_kwargs or {}),
    )
    out = np.concatenate([res.results[i]["v"] for i in range(NCORES)], axis=0)
    return out, res


def kernel(**inputs):
    x = inputs["x"]
    W = inputs["W"]
    out, _ = run(x, W, trace=False)
    return out



# revision 2
# speedup vs baseline: 100.8041x; 100.8041x over previous
# Trainium2 Bass kernel for nn_CapsuleLayer (dynamic-routing capsule layer).
#
# Math reformulation (exact, not approximate):
#   einsum('bni,njkl->bnjl', x, W) contracts i and k independently, so
#     predictions[b,n,j,l] = xs[b,n] * Ws[n,j,l]
#   with xs = x.sum(-1) [B,N], Ws = W.sum(2) [N,J,L].
#   Routing logits b start at 0 and update b += xs * (Ws . v), which is linear
#   in v, so the [B,N,J] logit state never needs to be materialized:
#     c_i = softmax_j( xs[b,n] * (Ws[n,j,:] . Vacc[b,j,:]) ),  Vacc = sum of past v's
#   Each iteration:
#     u[b,n,j] = xs[b,n] * c[b,n,j]
#     s[b,j,l] = sum_n u[b,n,j] * Ws[n,j,l]      (PE cross-product + diag extract)
#     v = squash(s)
#
# Sharding: only Ws = W.sum(k) [N,J,L] (4.7MB) is ever needed by routing, so
# the 37.7MB W read is sharded over N across the 8 cores (4.7MB each); each
# core k-reduces its shard on-chip, a 590KB/rank AllGather replicates the
# reduced Ws, and routing then runs fully data-parallel over B (8 samples
# per core, no further communication).  Per-core HBM traffic drops from
# ~38MB (W replicated) to ~11MB.
import numpy as np

import concourse.bass as bass
import concourse.mybir as mybir
import concourse.tile as tile
from concourse import bacc
from concourse import bass_utils
from concourse.bass import ts
from concourse.masks import make_identity

B, N, DI, J, L = 64, 2304, 8, 32, 16
NCORES = 8
BC = B // NCORES          # 8 samples per core
NS = N // NCORES          # 288 W rows per core
CH = 96                   # phase-1 k-reduce chunk rows
NCH = NS // CH            # 3 chunks
P = 128
NT = N // P               # 18 n-tiles
JL = J * L                # 512
BJ = BC * J               # 256
NG = JL // P              # 4 partition groups of the (j,l) axis
EPS = 1e-7

F32 = mybir.dt.float32
F32R = mybir.dt.float32r
AX = mybir.AxisListType
AF = mybir.ActivationFunctionType
ALU = mybir.AluOpType


def r(ap):
    """bitcast to float32r for full-rate PE matmuls (moving dim >= 256)."""
    return ap.bitcast(F32R)


def _emit(ctx, tc, x_ap, w_ap, v_ap, stage=5):
    nc = tc.nc

    # ---------------- pools ----------------
    const = ctx.enter_context(tc.tile_pool(name="const", bufs=1))
    wstream = ctx.enter_context(tc.tile_pool(name="wstream", bufs=3))
    xio = ctx.enter_context(tc.tile_pool(name="xio", bufs=2))
    work = ctx.enter_context(tc.tile_pool(name="work", bufs=4))
    small = ctx.enter_context(tc.tile_pool(name="small", bufs=4))
    tailp = ctx.enter_context(tc.tile_pool(name="tailp", bufs=2))
    dram = ctx.enter_context(tc.tile_pool(name="dram", bufs=1, space="DRAM"))
    ps_wT = ctx.enter_context(tc.tile_pool(name="ps_wT", bufs=1, space="PSUM"))
    ps_A = ctx.enter_context(tc.tile_pool(name="ps_A", bufs=3, space="PSUM"))
    ps_S = ctx.enter_context(tc.tile_pool(name="ps_S", bufs=1, space="PSUM"))
    ps_vT = ctx.enter_context(tc.tile_pool(name="ps_vT", bufs=1, space="PSUM"))

    # ---------------- constants ----------------
    id128 = const.tile([P, P], F32, name="id128")
    make_identity(nc, id128)
    id128r = const.tile([P, P], F32R, name="id128r")
    nc.vector.tensor_copy(out=id128r, in_=id128)

    # Mdiag[p, 0, k] = 1.0 iff k == p % 32   (diagonal-block extraction mask)
    Mdiag = const.tile([P, 1, 1, J], F32, name="Mdiag")
    nc.gpsimd.memset(Mdiag, 0.0)
    for q in range(P // J):
        nc.gpsimd.affine_select(
            out=Mdiag[ts(q, J), 0, 0, :],
            in_=Mdiag[ts(q, J), 0, 0, :],
            compare_op=ALU.not_equal,
            fill=1.0,
            base=0,
            pattern=[[-1, J]],
            channel_multiplier=1,
        )

    # maskbd[p, g, j] = 1.0 iff j == g*8 + p//16  (block-diag v builder),
    # i.e. iff (p + 128g - 16j) in [0, 15]. Built arithmetically because
    # engine ops require 32-aligned partition starts.
    jiota = const.tile([P, 1, J], F32, name="jiota")
    nc.gpsimd.iota(
        out=jiota,
        pattern=[[0, 1], [1, J]],
        base=0,
        channel_multiplier=0,
        allow_small_or_imprecise_dtypes=True,
    )
    piota = const.tile([P, 1, 1], F32, name="piota")
    nc.gpsimd.iota(
        out=piota,
        pattern=[[0, 1]],
        base=0,
        channel_multiplier=1,
        allow_small_or_imprecise_dtypes=True,
    )
    maskbd = const.tile([P, NG, 1, J], F32, name="maskbd")
    pg = const.tile([P, 1, 1], F32, name="pg")
    q_t = const.tile([P, 1, J], F32, name="q_t")
    qa = const.tile([P, 1, J], F32, name="qa")
    qb = const.tile([P, 1, J], F32, name="qb")
    for g in range(NG):
        nc.vector.tensor_scalar_add(out=pg, in0=piota, scalar1=float(P * g))
        nc.vector.tensor_scalar(
            out=q_t, in0=jiota, scalar1=-float(L), scalar2=pg,
            op0=ALU.mult, op1=ALU.add,
        )
        nc.vector.tensor_scalar(
            out=qa, in0=q_t, scalar1=0.0, scalar2=None, op0=ALU.is_ge
        )
        nc.vector.tensor_scalar(
            out=qb, in0=q_t, scalar1=float(L - 1), scalar2=None, op0=ALU.is_le
        )
        nc.vector.tensor_mul(out=maskbd[:, g], in0=qa, in1=qb)

    eps_ap = const.tile([P, 1], F32, name="eps_ap")
    nc.gpsimd.memset(eps_ap, EPS)

    # REP[p, (jm l)] = 1.0 iff l == p  (partition-replication stationary matrix:
    # out = REP.T @ rhs copies rhs's 16 partitions to all 8 16-partition groups)
    REP = const.tile([L, P], F32, name="REP")
    nc.gpsimd.memset(REP, 0.0)
    nc.gpsimd.affine_select(
        out=REP.rearrange("p (jm l) -> p jm l", l=L),
        in_=REP.rearrange("p (jm l) -> p jm l", l=L),
        compare_op=ALU.not_equal,
        fill=1.0,
        base=0,
        pattern=[[0, P // L], [1, L]],
        channel_multiplier=-1,
    )
    REPr = const.tile([L, P], F32R, name="REPr")
    nc.vector.tensor_copy(out=REPr, in_=REP)

    # ---------------- persistent tensors ----------------
    xs = const.tile([P, NT, BC, 1], F32, name="xs")        # xs[n%128, n//128, b]
    Ws = const.tile([P, NT, JL], F32R, name="Ws")           # Ws[n%128, n//128, (j l)]
    WsT = const.tile([P, NG, N], F32R, name="WsT")          # WsT[(j l)%128, (j l)//128, n]
    Vrep = const.tile([P, BJ], F32, name="Vrep")           # Vacc[l, (b j)] replicated x8 over partitions

    # ---------------- phase 1: own W shard -> k-reduced Ws shard in DRAM ----
    ws_in = dram.tile([NS, JL], F32, name="ws_in")
    ws_all = dram.tile([N, JL], F32, name="ws_all", addr_space="Shared")

    def w_reduce_chunk(c):
        w_c = wstream.tile([CH, J, DI, L], F32, name="w_c", tag="w_c")
        nc.sync.dma_start(out=w_c, in_=w_ap[ts(c, CH)])
        ws_c = wstream.tile([CH, J, L], F32, name="ws_c", tag="ws_c")
        JH = J // 2
        # DVE: strided-view reduce on the first half of j
        nc.vector.reduce_sum(
            out=ws_c[:, 0:JH, :],
            in_=w_c[:, 0:JH].rearrange("p j k l -> p j l k"),
            axis=AX.X,
        )
        # POOL: in-place add-tree on the second half
        wh = w_c[:, JH:J]
        nc.gpsimd.tensor_add(
            out=wh[:, :, 0 : DI // 2, :],
            in0=wh[:, :, 0 : DI // 2, :],
            in1=wh[:, :, DI // 2 : DI, :],
        )
        nc.gpsimd.tensor_add(
            out=wh[:, :, 0 : DI // 4, :],
            in0=wh[:, :, 0 : DI // 4, :],
            in1=wh[:, :, DI // 4 : DI // 2, :],
        )
        nc.gpsimd.tensor_add(
            out=ws_c[:, JH:J, :], in0=wh[:, :, 0, :], in1=wh[:, :, 1, :]
        )
        nc.sync.dma_start(
            out=ws_in[ts(c, CH), :],
            in_=ws_c.rearrange("p j l -> p (j l)"),
        )

    # ---------------- x prep: xs[n, b] = sum_i x[b, n, i] ----------------
    # x lands directly in [n-part, b, i] layout via a strided DMA (32B
    # contiguous runs); the i-reduce is then a cheap full-width DVE op.
    def x_prep_tile(t):
        x_t = xio.tile([P, BC, DI], F32, name="x_t", tag="x_t")
        nc.sync.dma_start(
            out=x_t, in_=x_ap.rearrange("b (t p) i -> p t b i", p=P)[:, t]
        )
        nc.vector.reduce_sum(out=xs[:, t], in_=x_t, axis=AX.X)

    # ---------------- phase 2: stream gathered Ws, transpose, iter-1 psS ----
    def phase2_tile(t, psS):
        nc.sync.dma_start(out=Ws[:, t, :].bitcast(F32), in_=ws_all[ts(t, P), :])
        wT_ps = ps_wT.tile([P, JL], F32R, name="wT_ps", tag="ps_share")
        for g in range(NG):
            nc.tensor.transpose(
                out=wT_ps[:, ts(g, P)], in_=Ws[:, t, ts(g, P)], identity=id128r
            )
        nc.scalar.copy(
            out=WsT[:, :, ts(t, P)],
            in_=wT_ps.rearrange("p (g n) -> p g n", g=NG),
        )
        # iter-1: c is uniform 1/J, so u = xs / J broadcast over j
        u = work.tile([P, BJ], F32R, name="u", tag="u")
        nc.vector.tensor_scalar(
            out=u.rearrange("p (b j) -> p b j", b=BC),
            in0=xs[:, t].to_broadcast([P, BC, J]),
            scalar1=1.0 / J,
            scalar2=None,
            op0=ALU.mult,
        )
        for h in range(2):
            nc.tensor.matmul(
                psS[h],
                lhsT=u[:, ts(h, P)],
                rhs=Ws[:, t, :],
                start=(t == 0),
                stop=(t == NT - 1),
            )

    # ---------------- routing iterations 2..3 (matmuls + softmax) ----------
    def routing_iter(it, bd):
        """Returns psS pair accumulated over all n-tiles for iteration `it`."""
        psS = [
            ps_S.tile([P, JL], F32, name=f"psS{h}_{it}", tag=f"psS{h}")
            for h in range(2)
        ]
        for t in range(NT):
            psA = ps_A.tile([P, BC, J], F32, name="psA", tag="psA")
            for g in range(NG):
                nc.tensor.matmul(
                    psA,
                    lhsT=WsT[:, g, ts(t, P)],
                    rhs=bd[:, g, :, :],
                    start=(g == 0),
                    stop=(g == NG - 1),
                )
            # logits = xs * A ; c = softmax_j ; u = xs * c
            Lt = work.tile([P, BJ], F32, name="Lt", tag="Lt")
            nc.vector.tensor_mul(
                out=Lt.rearrange("p (b j) -> p b j", b=BC),
                in0=psA,
                in1=xs[:, t].to_broadcast([P, BC, J]),
            )
            Et = work.tile([P, BJ], F32, name="Et", tag="Et")
            nc.scalar.activation(out=Et, in_=Lt, func=AF.Exp)
            St = small.tile([P, BC, 1], F32, name="St", tag="St")
            nc.vector.reduce_sum(
                out=St, in_=Et.rearrange("p (b j) -> p b j", b=BC), axis=AX.X
            )
            Rt = small.tile([P, BC, 1], F32, name="Rt", tag="Rt")
            nc.vector.reciprocal(out=Rt, in_=St)
            xsR = small.tile([P, BC, 1], F32, name="xsR", tag="xsR")
            nc.vector.tensor_mul(out=xsR, in0=Rt, in1=xs[:, t])
            u = work.tile([P, BJ], F32R, name="u", tag="u")
            nc.gpsimd.tensor_mul(
                out=u.rearrange("p (b j) -> p b j", b=BC),
                in0=Et.rearrange("p (b j) -> p b j", b=BC),
                in1=xsR.to_broadcast([P, BC, J]),
            )
            for h in range(2):
                nc.tensor.matmul(
                    psS[h],
                    lhsT=u[:, ts(h, P)],
                    rhs=Ws[:, t, :],
                    start=(t == 0),
                    stop=(t == NT - 1),
                )
        return psS

    # ---------------- iteration tail: diag extract + squash ----------------
    def iter_tail(psS):
        """Returns v_a [P, 2, L] where row p of half h holds v[b, j, :] with
        b = 4*h + p//32, j = p % 32."""
        s_a = tailp.tile([P, 2, L], F32, name="s_a", tag="s_a")
        for h in range(2):
            dtmp = tailp.tile([P, L, J], F32, name="dtmp", tag="dtmp")
            nc.vector.tensor_mul(
                out=dtmp,
                in0=psS[h].rearrange("p (k l) -> p l k", k=J),
                in1=Mdiag[:, 0].to_broadcast([P, L, J]),
            )
            nc.vector.reduce_sum(out=s_a[:, h, :], in_=dtmp, axis=AX.X)

        # ---- squash: v = s * n/(1+n)/sqrt(n+eps) ----
        nrm = tailp.tile([P, 2, 1], F32, name="nrm", tag="nrm")
        sq = tailp.tile([P, 2, L], F32, name="sq", tag="sq")
        nc.vector.tensor_mul(out=sq, in0=s_a, in1=s_a)
        nc.vector.reduce_sum(out=nrm, in_=sq, axis=AX.X)
        d1 = tailp.tile([P, 2, 1], F32, name="d1", tag="d1")
        nc.vector.tensor_scalar_add(out=d1, in0=nrm, scalar1=1.0)
        sqt = tailp.tile([P, 2, 1], F32, name="sqt", tag="sqt")
        nc.scalar.activation(out=sqt, in_=nrm, func=AF.Sqrt, bias=eps_ap)
        den = tailp.tile([P, 2, 1], F32, name="den", tag="den")
        nc.vector.tensor_mul(out=den, in0=d1, in1=sqt)
        rec = tailp.tile([P, 2, 1], F32, name="rec", tag="rec")
        nc.vector.reciprocal(out=rec, in_=den)
        # v = (s * rec) * nrm, fused per half via dual-scalar tensor_scalar
        v_a = tailp.tile([P, 2, L], F32R, name="v_a", tag="v_a")
        for h in range(2):
            nc.vector.tensor_scalar(
                out=v_a[:, h, :],
                in0=s_a[:, h, :],
                scalar1=rec[:, h, :],
                scalar2=nrm[:, h, :],
                op0=ALU.mult,
                op1=ALU.mult,
            )
        return v_a

    def accumulate_v(v_a, first):
        """Transpose v_a into vT[l, (b j)] (replicated over 8 partition groups)
        and accumulate into Vrep; build block-diag bd for the next iteration."""
        vT = ps_vT.tile([L, BJ], F32R, name="vT", tag="vT")
        for h in range(2):
            nc.tensor.transpose(
                out=vT[:, ts(h, P)], in_=v_a[:, h, :], identity=id128r
            )
        vT_sb = tailp.tile([L, BJ], F32R, name="vT_sb", tag="vT_sb")
        nc.vector.tensor_copy(out=vT_sb, in_=vT)
        vrep_ps = ps_wT.tile([P, BJ], F32, name="vrep_ps", tag="ps_share")
        nc.tensor.matmul(vrep_ps, lhsT=REPr, rhs=vT_sb, start=True, stop=True)
        if first:
            nc.vector.tensor_copy(out=Vrep, in_=vrep_ps)
        else:
            nc.vector.tensor_add(out=Vrep, in0=Vrep, in1=vrep_ps)
        bd = tailp.tile([P, NG, BC, J], F32R, name="bd", tag="bd")
        for g in range(NG):
            nc.gpsimd.tensor_mul(
                out=bd[:, g, :, :],
                in0=Vrep.rearrange("p (b j) -> p b j", b=BC),
                in1=maskbd[:, g].to_broadcast([P, BC, J]),
            )
        return bd

    # ---------------- main schedule ----------------
    v_flat = v_ap.rearrange("b j l -> (b j) l")

    def emit_out(v_x):
        for h in range(2):
            nc.sync.dma_start(out=v_flat[ts(h, P)], in_=v_x[:, h, :].bitcast(F32))

    def one_pass():
        for c in range(NCH):
            w_reduce_chunk(c)
        nc.gpsimd.collective_compute(
            "AllGather",
            ALU.bypass,
            replica_groups=[list(range(NCORES))],
            ins=[ws_in[:].opt()],
            outs=[ws_all[:].opt()],
        )
        for t in range(NT):
            x_prep_tile(t)
        psS1 = [
            ps_S.tile([P, JL], F32, name=f"psS{h}_1", tag=f"psS{h}")
            for h in range(2)
        ]
        for t in range(NT):
            phase2_tile(t, psS1)
        v1 = iter_tail(psS1)
        bd1 = accumulate_v(v1, first=True)
        v2 = iter_tail(routing_iter(2, bd1))
        bd2 = accumulate_v(v2, first=False)
        v3 = iter_tail(routing_iter(3, bd2))
        emit_out(v3)

    if stage >= 100:
        for i in range(stage - 100):
            if i:
                tc.strict_bb_all_engine_barrier()
            one_pass()
    else:
        one_pass()


_nc_cache = {}


def build(stage=5):
    if stage not in _nc_cache:
        from contextlib import ExitStack

        nc = bacc.Bacc(
            "TRN2", target_bir_lowering=False, debug=False, num_devices=NCORES
        )
        x_ap = nc.dram_tensor("x", [BC, N, DI], F32, kind="ExternalInput").ap()
        w_ap = nc.dram_tensor("w", [NS, J, DI, L], F32, kind="ExternalInput").ap()
        v_ap = nc.dram_tensor("v", [BC, J, L], F32, kind="ExternalOutput").ap()
        with (
            tile.TileContext(nc) as tc,
            ExitStack() as ctx,
            nc.allow_low_precision(
                reason="f32r is a rounded fp32 view required for full-rate PE "
                "matmuls; accumulation still happens in fp32 PSUM"
            ),
        ):
            _emit(ctx, tc, x_ap, w_ap, v_ap, stage=stage)
        nc.compile()
        _nc_cache[stage] = nc
    return _nc_cache[stage]


def make_in_maps(x, W):
    x = np.ascontiguousarray(np.asarray(x, dtype=np.float32))
    W = np.ascontiguousarray(np.asarray(W, dtype=np.float32))
    assert x.shape == (B, N, DI) and W.shape == (N, J, DI, L)
    return [
        {"x": x[i * BC : (i + 1) * BC], "w": W[i * NS : (i + 1) * NS]}
        for i in range(NCORES)
    ]


def run(x, W, trace=False, trace_kwargs=None):
    nc = build()
    in_maps = make_in_maps(x, W)
    res = bass_utils.run_bass_kernel_spmd(
        nc,
        in_maps,
        core_ids=list(range(NCORES)),
        trace=trace,
        **(trace_kwargs or {}),
    )
    out = np.concatenate([res.results[i]["v"] for i in range(NCORES)], axis=0)
    return out, res


def kernel(**inputs):
    x = inputs["x"]
    W = inputs["W"]
    out, _ = run(x, W, trace=False)
    return out
